# revision 2
# baseline (speedup 1.0000x reference)
"""Bass/Tile TRN2 kernel: 16-head self-attention (B=4, S=2048, D=1024, H=16).

Sharding over 8 NeuronCores: core c = (batch b = c//2, head-half hh = c%2).
Each core:
  - QKV projection for its 8 heads on its batch (x[b] @ W_qkv[:, slice] + b)
  - full (non-causal) attention for those 8 heads
  - partial output projection: attn_heads @ W_out[hh*512:(hh+1)*512, :]
Host gathers: out[b] = partial[2b] + partial[2b+1]  (b_out folded into even core).

Device-side layout choices (all matmuls transpose-free):
  - x is fed pre-transposed per batch: xT [D, S] (d_model on partitions).
  - Q^T, K^T computed as [feat, seq] (psum = W_slice.T @ xT)  -> scores
    lhsT/rhs directly.
  - scores computed transposed: S^T[j, i] = K_h^T.T @ Q_h^T, so softmax sum
    is a matmul with an appended ones column and exp is a single ScalarE
    activation (scale=1/sqrt(dk) folded in). Scores are in [-3, 3] for this
    problem so no max-subtraction is needed.
  - AV: psum[dk(+pad), i] = Vpad_h.T @ exp(S^T); Vpad places V columns at
    0:64 (even local head) or 64:128 (odd local head) and a ones column on
    the opposite side, so the unnormalized head outputs land partition-
    aligned for the 2-head-per-128-partition stacking the output projection
    needs, and the softmax row-sums ride along for free.
  - normalization: reciprocal of rowsum, partition-broadcast via a DRAM
    bounce DMA, one elementwise multiply.
"""

import sys

import numpy as np

if "/opt/trn_rl_repo" not in sys.path:
    sys.path.insert(0, "/opt/trn_rl_repo")

import ml_dtypes

B = 4
D_MODEL = 1024
NUM_HEADS = 16
DK = 64
P = 128
F = 512          # per-core q/k/v feature slice (8 heads * 64)
DC = D_MODEL // P  # 8 d_model chunks
FC = F // P        # 4 feature chunks
HPC = 8            # heads per core
N_CORES = 8
BF16 = ml_dtypes.bfloat16

_NC_CACHE = {}


def _build(S=2048, IB=1024, debug=False):
    from contextlib import ExitStack

    import concourse.bacc as bacc
    import concourse.bass as bass
    import concourse.mybir as mybir
    import concourse.tile as tile

    f32 = mybir.dt.float32
    bf16 = mybir.dt.bfloat16
    AF = mybir.ActivationFunctionType
    OP = mybir.AluOpType

    SC = S // P        # seq chunks of 128
    NIB = S // IB      # attention i-blocks
    NH = IB // 512     # 512-wide matmul chunks per i-block

    nc = bacc.Bacc(
        "TRN2", target_bir_lowering=False, debug=debug, num_devices=N_CORES
    )

    xT = nc.dram_tensor("xT", [D_MODEL, S], bf16, kind="ExternalInput")
    wq = nc.dram_tensor("wq", [D_MODEL, F], bf16, kind="ExternalInput")
    wk = nc.dram_tensor("wk", [D_MODEL, F], bf16, kind="ExternalInput")
    wv = nc.dram_tensor("wv", [D_MODEL, F], bf16, kind="ExternalInput")
    bqk = nc.dram_tensor("bqk", [P, 2 * FC], f32, kind="ExternalInput")
    bv = nc.dram_tensor("bv", [P, F], f32, kind="ExternalInput")
    wo = nc.dram_tensor("wo", [F, D_MODEL], bf16, kind="ExternalInput")
    bo = nc.dram_tensor("bo", [P, D_MODEL], f32, kind="ExternalInput")
    out = nc.dram_tensor("out", [S, D_MODEL], f32, kind="ExternalOutput")

    with tile.TileContext(nc) as tc, ExitStack() as ctx:
        consts = ctx.enter_context(tc.tile_pool(name="consts", bufs=1))
        psum = ctx.enter_context(tc.tile_pool(name="psum", bufs=2, space="PSUM"))
        pts = ctx.enter_context(tc.tile_pool(name="pts", bufs=3))
        drains = ctx.enter_context(tc.tile_pool(name="drains", bufs=3))
        outs = ctx.enter_context(tc.tile_pool(name="outs", bufs=3))
        dram = ctx.enter_context(tc.tile_pool(name="dram", bufs=3, space="DRAM"))

        # ---- persistent SBUF tensors ----
        xT_sb = consts.tile([P, DC, S], bf16, tag="xT_sb")
        wq_sb = consts.tile([P, DC, F], bf16, tag="wq_sb")
        wk_sb = consts.tile([P, DC, F], bf16, tag="wk_sb")
        wv_sb = consts.tile([P, DC, F], bf16, tag="wv_sb")
        bqk_sb = consts.tile([P, 2 * FC], f32, tag="bqk_sb")
        bv_sb = consts.tile([P, F], f32, tag="bv_sb")
        wo_sb = consts.tile([P, FC, D_MODEL], bf16, tag="wo_sb")
        bo_sb = consts.tile([P, D_MODEL], f32, tag="bo_sb")
        qt_sb = consts.tile([P, FC, S], bf16, tag="qt_sb")
        kt_sb = consts.tile([P, FC, S], bf16, tag="kt_sb")
        v_sb = consts.tile([P, SC, HPC, P], bf16, tag="v_sb")
        ao_sb = consts.tile([P, FC, S], bf16, tag="ao_sb")

        sync = nc.sync
        sync.dma_start(out=xT_sb, in_=xT.ap().rearrange("(n p) s -> p n s", p=P))
        sync.dma_start(out=wv_sb, in_=wv.ap().rearrange("(n p) f -> p n f", p=P))
        sync.dma_start(out=wq_sb, in_=wq.ap().rearrange("(n p) f -> p n f", p=P))
        sync.dma_start(out=wk_sb, in_=wk.ap().rearrange("(n p) f -> p n f", p=P))
        sync.dma_start(out=bqk_sb, in_=bqk.ap())
        sync.dma_start(out=bv_sb, in_=bv.ap())
        sync.dma_start(out=wo_sb, in_=wo.ap().rearrange("(n p) f -> p n f", p=P))
        sync.dma_start(out=bo_sb, in_=bo.ap())

        # ---- V tile: zeros, then ones columns (even head -> col 64,
        # odd head -> col 0), V data filled by projection below ----
        nc.vector.memset(v_sb, 0.0)
        for hl in range(HPC):
            one_col = DK if hl % 2 == 0 else 0
            nc.vector.memset(v_sb[:, :, hl, one_col : one_col + 1], 1.0)

        # ---- V projection: psum[seq128, feat512] = xT_chunk.T @ Wv ----
        for sc in range(SC):
            ps = psum.tile([P, IB], f32, tag="sc")
            pv = ps[:, 0:F]
            for dc in range(DC):
                nc.tensor.matmul(
                    pv,
                    lhsT=xT_sb[:, dc, sc * P : (sc + 1) * P],
                    rhs=wv_sb[:, dc, :],
                    start=(dc == 0),
                    stop=(dc == DC - 1),
                )
            # bias-add + scatter into per-head slots of v_sb
            pv3 = pv.rearrange("p (m two d) -> p m two d", two=2, d=DK)
            bv3 = bv_sb.rearrange("p (m two d) -> p m two d", two=2, d=DK)
            v4 = v_sb[:, sc]  # [P, HPC, P]
            v4r = v4.rearrange("p (m two) c -> p m two c", two=2)
            nc.vector.tensor_tensor(
                out=v4r[:, :, 0, 0:DK], in0=pv3[:, :, 0, :], in1=bv3[:, :, 0, :],
                op=OP.add,
            )
            nc.vector.tensor_tensor(
                out=v4r[:, :, 1, DK:P], in0=pv3[:, :, 1, :], in1=bv3[:, :, 1, :],
                op=OP.add,
            )

        # ---- Q^T / K^T projection: psum[feat128, seq512] = W_chunk.T @ xT ----
        for t in range(2):
            w_sb = (wq_sb, wk_sb)[t]
            dest = (qt_sb, kt_sb)[t]
            for fc in range(FC):
                for icb in range(S // 512):
                    ps = psum.tile([P, IB], f32, tag="sc")
                    pq = ps[:, 0:512]
                    for dc in range(DC):
                        nc.tensor.matmul(
                            pq,
                            lhsT=w_sb[:, dc, fc * P : (fc + 1) * P],
                            rhs=xT_sb[:, dc, icb * 512 : (icb + 1) * 512],
                            start=(dc == 0),
                            stop=(dc == DC - 1),
                        )
                    nc.vector.tensor_scalar_add(
                        out=dest[:, fc, icb * 512 : (icb + 1) * 512],
                        in0=pq,
                        scalar1=bqk_sb[:, t * FC + fc : t * FC + fc + 1],
                    )

        # ---- attention, head by head ----
        for hl in range(HPC):
            par = hl % 2
            ko = DK * par           # partition offset of this head in qt/kt
            fcq = hl // 2           # feature chunk in qt/kt; also ao_sb chunk
            rows = slice(0, DK) if par == 0 else slice(DK, P)
            rsr = DK if par == 0 else 0   # rowsum partition in AV psum

            for ib in range(NIB):
                i0 = ib * IB
                po = psum.tile([P, IB], f32, tag="av")

                def sc_exp(jc):
                    ps = psum.tile([P, IB], f32, tag="sc")
                    for h2 in range(NH):
                        nc.tensor.matmul(
                            ps[:, h2 * 512 : (h2 + 1) * 512],
                            lhsT=kt_sb[ko : ko + DK, fcq, jc * P : (jc + 1) * P],
                            rhs=qt_sb[
                                ko : ko + DK, fcq,
                                i0 + h2 * 512 : i0 + (h2 + 1) * 512,
                            ],
                            start=True,
                            stop=True,
                        )
                    pt = pts.tile([P, IB], bf16, tag="pt")
                    nc.scalar.activation(pt, ps, AF.Exp, scale=0.125)
                    return pt

                def av(jc, pt):
                    for h2 in range(NH):
                        nc.tensor.matmul(
                            po[:, h2 * 512 : (h2 + 1) * 512],
                            lhsT=v_sb[:, jc, hl, :],
                            rhs=pt[:, h2 * 512 : (h2 + 1) * 512],
                            start=(jc == 0),
                            stop=(jc == SC - 1),
                        )

                prev = sc_exp(0)
                for jc in range(1, SC):
                    cur = sc_exp(jc)
                    av(jc - 1, prev)
                    prev = cur
                av(SC - 1, prev)

                # drain: unnormalized head output + rowsum, then normalize
                ao_dest = ao_sb[rows, fcq, i0 : i0 + IB]
                nc.vector.tensor_copy(out=ao_dest, in_=po[rows, :])
                rs_t = drains.tile([P, IB], f32, tag="rs")
                nc.vector.tensor_copy(
                    out=rs_t[rsr : rsr + 1, :], in_=po[rsr : rsr + 1, :]
                )
                rr_t = drains.tile([P, IB], f32, tag="rr")
                nc.vector.reciprocal(
                    out=rr_t[rsr : rsr + 1, :], in_=rs_t[rsr : rsr + 1, :]
                )
                dscr = dram.tile([1, IB], f32, tag="dscr")
                sync.dma_start(out=dscr, in_=rr_t[rsr : rsr + 1, :])
                rbc = drains.tile([P, IB], f32, tag="rbc")
                bcast_src = bass.AP(
                    tensor=dscr.tensor, offset=dscr.offset, ap=[[0, DK], [1, IB]]
                )
                sync.dma_start(out=rbc[rows, :], in_=bcast_src)
                nc.vector.tensor_tensor(
                    out=ao_dest, in0=ao_dest, in1=rbc[rows, :], op=OP.mult
                )

        # ---- output projection: psum[i128, out1024] += aoT.T @ Wo ----
        for ic in range(SC):
            ps = psum.tile([P, D_MODEL], f32, tag="av")
            for hfc in range(FC):
                for nb in range(D_MODEL // 512):
                    nc.tensor.matmul(
                        ps[:, nb * 512 : (nb + 1) * 512],
                        lhsT=ao_sb[:, hfc, ic * P : (ic + 1) * P],
                        rhs=wo_sb[:, hfc, nb * 512 : (nb + 1) * 512],
                        start=(hfc == 0),
                        stop=(hfc == FC - 1),
                    )
            o_t = outs.tile([P, D_MODEL], f32, tag="o_t")
            nc.vector.tensor_tensor(out=o_t, in0=ps, in1=bo_sb, op=OP.add)
            sync.dma_start(out=out.ap()[ic * P : (ic + 1) * P, :], in_=o_t)

    nc.compile()
    return nc


def _get_nc(S=2048, IB=1024, debug=False):
    key = (S, IB, debug)
    if key not in _NC_CACHE:
        _NC_CACHE[key] = _build(S, IB, debug)
    return _NC_CACHE[key]


def make_in_maps(x, W_qkv, b_qkv, W_out, b_out):
    x = np.asarray(x, dtype=np.float32)
    W_qkv = np.asarray(W_qkv, dtype=np.float32)
    b_qkv = np.asarray(b_qkv, dtype=np.float32)
    W_out = np.asarray(W_out, dtype=np.float32)
    b_out = np.asarray(b_out, dtype=np.float32)
    S = x.shape[1]

    xTs = [np.ascontiguousarray(x[b].T).astype(BF16) for b in range(B)]
    per_hh = []
    for hh in range(2):
        qs = slice(hh * F, hh * F + F)
        ks = slice(D_MODEL + hh * F, D_MODEL + hh * F + F)
        vs = slice(2 * D_MODEL + hh * F, 2 * D_MODEL + hh * F + F)
        d = {
            "wq": W_qkv[:, qs].astype(BF16),
            "wk": W_qkv[:, ks].astype(BF16),
            "wv": W_qkv[:, vs].astype(BF16),
            "bqk": np.ascontiguousarray(
                np.concatenate(
                    [b_qkv[qs].reshape(FC, P).T, b_qkv[ks].reshape(FC, P).T],
                    axis=1,
                )
            ).astype(np.float32),
            "bv": np.ascontiguousarray(
                np.broadcast_to(b_qkv[vs], (P, F))
            ).astype(np.float32),
            "wo": np.ascontiguousarray(W_out[hh * F : (hh + 1) * F, :]).astype(
                BF16
            ),
            "bo": (
                np.ascontiguousarray(np.broadcast_to(b_out, (P, D_MODEL))).astype(
                    np.float32
                )
                if hh == 0
                else np.zeros((P, D_MODEL), dtype=np.float32)
            ),
        }
        per_hh.append(d)

    maps = []
    for c in range(N_CORES):
        b, hh = divmod(c, 2)
        m = dict(per_hh[hh])
        m["xT"] = xTs[b]
        maps.append(m)
    return maps


def gather(results):
    outs = [np.asarray(r["out"], dtype=np.float32) for r in results]
    return np.stack([outs[2 * b] + outs[2 * b + 1] for b in range(B)], axis=0)


def run(in_maps, trace=False, S=2048):
    from concourse.bass_utils import run_bass_kernel_spmd

    nc = _get_nc(S=S)
    kw = {}
    if trace:
        kw = {"trace": True, "trace_cores": [0]}
    res = run_bass_kernel_spmd(nc, in_maps, core_ids=list(range(N_CORES)), **kw)
    return res


def kernel(x, W_qkv, b_qkv, W_out, b_out):
    in_maps = make_in_maps(x, W_qkv, b_qkv, W_out, b_out)
    res = run(in_maps, S=np.asarray(x).shape[1])
    return gather(res.results)
